# revision 18
# baseline (speedup 1.0000x reference)
"""Trainium2 Bass kernel: single-head causal attention.

Problem: x[B=8,T=2048,C=1024] @ Wq/Wk/Wv[C,H=64] -> causal softmax attention
-> out[B,T,H].  Sharding: pure data-parallel over B, one batch element per
NeuronCore (8 cores, no collectives).

Layout strategy (per core):
  - host feeds x[b].T  (so the C contraction dim lands on SBUF partitions)
  - q,k projections: W-chunk stationary ([Wq*scale | Wk] packed to 128 cols)
    -> psum [128(q|k), 512]; one DVE copy puts q^T on partitions 0-63 of
    q_sb and one puts k^T on partitions 0-63 of k_sb
  - v projection: x-chunk stationary (flip_v) -> psum [128t, 64] lands v in
    natural [T,64] layout directly, no transposes (this beat W-stationary +
    PE transposes by ~4-6us measured); v_sb carries a ones column so att@v
    and the softmax row-sums come out of one accumulated matmul
  - scores are computed in sT layout [T_k, T_q]; diagonal 128-blocks are
    shortened to their live q-range; softmax uses exp without max-subtraction
    (|s| <~ 6 so fp32 exp is safe); causal mask is a single [128,128]
    staircase multiply on the boundary sub-block only (gpsimd)
  - score matmuls have 64-deep contraction, so k-chunks alternate PE row
    halves (tile_position) and pairs overlap in the array (~3us measured)
  - outT_aug [65, T_q] is evacuated on DVE (ACT copy measured slower once
    exp saturates ACT) and PE-transposed to [T_q, 65] per 128-row subtile;
    rows are scaled by 1/sum and DMA'd out per q-block (dma1=0)
  - all matmuls in bf16 (rel err 5.4e-3 vs 2e-2 budget)
  - q/k/v/out SBUF buffers are double-buffered; the last q-block's att@v
    runs at the start of the next rep (prev mechanism) to overlap its tail

Measured (differential For_i timing, 8 cores): 41.4us (session start,
W-stationary v) -> ~30.4us (flip_v + act_ot:0 + dma1:0).  Rejected by
measurement: mproj (+15us), vweave (+2us), weave/early_scores (+1-6us),
vpack (verifier reject), mask_dve, rotate, dve_dup:0 (+5us), expp 20.
Cost-model floor ~25.5us PE-busy; ACT exp ~19us busy (not critical:
no_act only saves 2us).
"""

import numpy as np

P = 128
B = 8
T = 2048
C = 1024
H = 64
QB = 512          # q-block width (score tile free dim)
NB = T // QB      # 4 q-blocks
CC = C // P       # 8 contraction chunks
KT = T // P       # 16 key tiles / T subtiles
N_CORES = 8

_CACHE = {}
CFG = {'flip_v': True, 'alt_scores': True, 'no_act': False, 'no_proj': False,
       'no_tail': False, 'bf16': True, 'dpix': False, 'bf16_out': True,
       'rotate': False, 'dma1': False, 'kbatch': True, 'mproj': False, 'bf16_dma': True, 'early_scores': False, 'weave': False, 'act_ot': False, 'dve_dup': True,
       'vpack': False, 'mask_dve': False, 'expp_bufs': 16, 'psS_bufs': 2,
       'psA_bufs': 2, 'psO_bufs': 2, 'vweave': False, 'vlate': False,
       'chunked': False}


def _build(reps=1, outer=0, cfg=None):
    import concourse.bacc as bacc
    import concourse.mybir as mybir
    import concourse.tile as tile
    from concourse.masks import make_identity

    cfg = dict(CFG, **(cfg or {}))
    dt = mybir.dt
    f32 = dt.float32
    f32r = dt.float32r
    bf16 = dt.bfloat16
    AF = mybir.ActivationFunctionType
    ALU = mybir.AluOpType

    mmdt = bf16 if CFG['bf16'] else f32r
    nc = bacc.Bacc(None, target_bir_lowering=False)
    xT_d = nc.dram_tensor("xT", [C, T], mmdt, kind="ExternalInput")
    nw = 3 * H if cfg['mproj'] else 2 * H
    wqk_d = nc.dram_tensor("wqk", [C, nw], mmdt, kind="ExternalInput")
    wv_d = nc.dram_tensor("wv", [C, H], mmdt, kind="ExternalInput")
    out_d = nc.dram_tensor("out", [T, H], bf16 if cfg['bf16_dma'] else f32,
                           kind="ExternalOutput")

    with tile.TileContext(nc) as tc:
        with (
            tc.tile_pool(name="consts", bufs=1) as consts,
            tc.tile_pool(name="xpool", bufs=1) as xpool,
            tc.tile_pool(name="qp", bufs=2) as qp,
            tc.tile_pool(name="kp", bufs=2) as kp,
            tc.tile_pool(name="vp", bufs=2) as vp,
            tc.tile_pool(name="expp", bufs=cfg['expp_bufs']) as expp,
            tc.tile_pool(name="otp", bufs=3) as otp,
            tc.tile_pool(name="sclp", bufs=4) as sclp,
            tc.tile_pool(name="outp", bufs=2) as outp,
            tc.tile_pool(name="psA", bufs=cfg['psA_bufs'], space="PSUM") as psA,
            tc.tile_pool(name="psS", bufs=cfg['psS_bufs'], space="PSUM") as psS,
            tc.tile_pool(name="psO", bufs=cfg['psO_bufs'], space="PSUM") as psO,
        ):
            ident = consts.tile([P, P], f32)
            make_identity(nc, ident)
            ident_mm = consts.tile([P, P], mmdt)
            nc.vector.tensor_copy(ident_mm, ident)
            # mask128[p, f] = 1.0 if f >= p else 0.0 (staircase for the
            # boundary 128x128 sub-block of each diagonal score tile)
            mask128 = consts.tile([P, P], mmdt)
            nc.gpsimd.memset(mask128, 1.0)
            nc.gpsimd.affine_select(
                out=mask128,
                in_=mask128,
                compare_op=ALU.is_ge,
                fill=0.0,
                base=0,
                pattern=[[1, P]],
                channel_multiplier=-1,
            )

            wqk_sb = consts.tile([P, CC, nw], mmdt)
            nc.sync.dma_start(wqk_sb[:], wqk_d[:, :].rearrange("(c p) h -> p c h", p=P))
            wv_sb = consts.tile([P, CC, H], mmdt)
            nc.sync.dma_start(wv_sb[:], wv_d[:, :].rearrange("(c p) h -> p c h", p=P))

            x_sb = xpool.tile([P, CC, T], mmdt)
            for bb in range(NB // 2):
                for c in range(CC):
                    nc.sync.dma_start(
                        x_sb[:, c, bb * 2 * QB:(bb + 1) * 2 * QB],
                        xT_d[c * P:(c + 1) * P, bb * 2 * QB:(bb + 1) * 2 * QB],
                    )

            ones_col = consts.tile([P, KT, 1], f32)
            nc.gpsimd.memset(ones_col[:], 1.0)

            if cfg['no_act']:
                et_const = consts.tile([P, 2, QB], mmdt)
                scr = consts.tile([P, 2, QB], f32)
                nc.gpsimd.memset(scr[:], 0.001)
                nc.vector.tensor_copy(et_const[:], scr[:])

            if cfg['no_proj']:
                scr2 = consts.tile([P, T // 2], f32)
                nc.gpsimd.memset(scr2[:], 0.01)
                qC = consts.tile([P, QB], mmdt)
                kC = consts.tile([P, T // 2], mmdt)
                vC = consts.tile([P, KT, H + 1], mmdt)
                nc.vector.tensor_copy(qC[:], scr2[:, 0:QB])
                nc.vector.tensor_copy(kC[:], scr2[:])
                for _t in range(KT):
                    nc.vector.tensor_copy(vC[:, _t, :], scr2[:, 0:H + 1])

            def make_bufs():
                if cfg['no_proj']:
                    out_sb = outp.tile([P, KT, H], bf16 if cfg['bf16_dma'] else f32, name="out_sb")
                    return None, kC, vC, out_sb
                # per-iteration rotating state: k history, v history
                if cfg['alt_scores']:
                    k_sb = kp.tile([P, T // 2], mmdt, name="k_sb")
                else:
                    k_sb = kp.tile([H, T], mmdt, name="k_sb")
                v_sb = vp.tile([P, KT, H + 1], mmdt, name="v_sb")
                nc.vector.tensor_copy(v_sb[:, :, H:H + 1], ones_col[:])
                out_sb = outp.tile([P, KT, H], bf16 if cfg['bf16_dma'] else f32, name="out_sb")
                return None, k_sb, v_sb, out_sb

            def project_mqkv(b, k_sb, v_sb):
                # one pass over x: q,k,v in natural [t, 3H] layout, then PE
                # transposes for q,k; v slices feed v_sb directly
                q_sb = qp.tile([P, QB], mmdt, name="q_sb")
                for r in range(2):
                    ps = psA.tile([P, 2, 3 * H], f32, tag="a", name="ps_m")
                    for tt in range(2):
                        t = b * NB + 2 * r + tt
                        tsl = slice(t * P, (t + 1) * P)
                        for c in range(CC):
                            nc.tensor.matmul(
                                ps[:, tt, :], x_sb[:, c, tsl], wqk_sb[:, c, :],
                                start=(c == 0), stop=(c == CC - 1),
                            )
                    nat = otp.tile([P, 2, 3 * H], mmdt, name="nat")
                    nc.vector.tensor_copy(nat, ps)
                    # v: natural already; strided copy into v_sb
                    nc.vector.tensor_copy(
                        v_sb[:, b * NB + 2 * r:b * NB + 2 * r + 2, 0:H],
                        nat[:, :, 2 * H:3 * H],
                    )
                    for tt in range(2):
                        t2 = 2 * r + tt
                        kc = 4 * b + t2
                        pq = psA.tile([H, 2, P], mmdt, tag="a", name="ps_tq")
                        nc.tensor.matmul(
                            pq[:, 0, :], nat[:, tt, 0:H],
                            ident_mm[:P, :P], is_transpose=True,
                        )
                        nc.tensor.matmul(
                            pq[:, 1, :], nat[:, tt, H:2 * H],
                            ident_mm[:P, :P], is_transpose=True,
                        )
                        nc.vector.tensor_copy(
                            q_sb[0:H, t2 * P:(t2 + 1) * P], pq[:, 0, :])
                        half = (kc % 2) * H
                        nc.vector.tensor_copy(
                            k_sb[half:half + H, (kc // 2) * P:(kc // 2 + 1) * P],
                            pq[:, 1, :],
                        )
                nc.gpsimd.tensor_copy(q_sb[H:P, :], q_sb[0:H, :])
                return q_sb

            def project_qk(b, k_sb, v_sb=None):
                if cfg['no_proj']:
                    return qC
                if cfg['alt_scores']:
                    q_sb = qp.tile([P, QB], mmdt, name="q_sb")
                else:
                    q_sb = qp.tile([H, QB], mmdt, name="q_sb")
                # [Wq*s|Wk] concatenated on host -> one M=128 matmul gives
                # qT on psum parts 0-63 and kT on parts 64-127
                bsl = slice(b * QB, (b + 1) * QB)
                ps = psA.tile([P, QB], f32, tag="a", name="ps_qk")
                psv = None
                if v_sb is not None:
                    # vweave: interleave the 32 x-stationary v matmuls (64-col
                    # streams) between the 8 long qk streams so each v
                    # LDWEIGHTS overlaps a 512-col qk matmul
                    psv = psA.tile([P, NB, H], f32, tag="a", name="ps_v")
                for c in range(CC):
                    nc.tensor.matmul(
                        ps, wqk_sb[:, c, :], x_sb[:, c, bsl],
                        start=(c == 0), stop=(c == CC - 1),
                    )
                    if psv is not None:
                        for s in range(NB):
                            t = b * NB + s
                            tsl = slice(t * P, (t + 1) * P)
                            nc.tensor.matmul(
                                psv[:, s, :], x_sb[:, c, tsl], wv_sb[:, c, :],
                                start=(c == 0), stop=(c == CC - 1),
                            )
                if psv is not None:
                    nc.vector.tensor_copy(
                        v_sb[:, b * NB:(b + 1) * NB, 0:H], psv[:])
                if cfg['alt_scores']:
                    # q duplicated on both partition halves; k chunks go to
                    # alternating halves so score matmuls ping-pong PE rows
                    nc.vector.tensor_copy(q_sb[0:H, :], ps[0:H, :])
                    if cfg['dve_dup']:
                        # all-bf16 SBUF copy hits DVE 4x mode and lands on an
                        # empty queue; on gpsimd it sat behind the mask queue
                        nc.vector.tensor_copy(q_sb[H:P, :], q_sb[0:H, :])
                    else:
                        nc.gpsimd.tensor_copy(q_sb[H:P, :], q_sb[0:H, :])
                    if cfg['kbatch']:
                        s4 = ps[H:P, :].rearrange("p (a k) -> p a k", k=P)
                        d4 = k_sb[:, 2 * b * P:(2 * b + 2) * P].rearrange(
                            "p (a k) -> p a k", k=P)
                        nc.vector.tensor_copy(d4[0:H, :, :], s4[:, 0::2, :])
                        nc.vector.tensor_copy(d4[H:P, :, :], s4[:, 1::2, :])
                    else:
                        for s in range(4):
                            kc = 4 * b + s
                            half = (kc % 2) * H
                            nc.vector.tensor_copy(
                                k_sb[half:half + H, (kc // 2) * P:(kc // 2 + 1) * P],
                                ps[H:P, s * P:(s + 1) * P],
                            )
                else:
                    nc.vector.tensor_copy(q_sb[:, :], ps[0:H, :])
                    nc.vector.tensor_copy(k_sb[:, bsl], ps[H:P, :])
                return q_sb

            def project_v(b, v_sb):
                if cfg['no_proj']:
                    return
                if cfg['vpack']:
                    # col-packed W-stationary: two c-chunks' Wv in disjoint
                    # col groups (partitions 0-63 / 64-127) stream their x
                    # chunks concurrently; halves summed on DVE afterwards
                    bsl = slice(b * QB, (b + 1) * QB)
                    ps = psA.tile([P, QB], f32, tag="a", name="ps_vp")
                    for cp in range(CC // 2):
                        nc.tensor.matmul(
                            ps[0:H, :], wv_sb[:, 2 * cp, :], x_sb[:, 2 * cp, bsl],
                            start=(cp == 0), stop=(cp == CC // 2 - 1),
                            tile_position=(0, 0),
                        )
                        nc.tensor.matmul(
                            ps[H:2 * H, :], wv_sb[:, 2 * cp + 1, :],
                            x_sb[:, 2 * cp + 1, bsl],
                            start=(cp == 0), stop=(cp == CC // 2 - 1),
                            tile_position=(0, 64),
                        )
                    vT = otp.tile([H, QB], mmdt, name="vT")
                    nc.vector.tensor_add(vT, ps[0:H, :], ps[H:2 * H, :])
                    pv = psA.tile([P, NB, H], mmdt, tag="a", name="ps_vn")
                    for s in range(NB):
                        nc.tensor.matmul(
                            pv[:, s, :],
                            vT[:, s * P:(s + 1) * P],
                            ident_mm[:H, :H],
                            is_transpose=True,
                        )
                    nc.vector.tensor_copy(v_sb[:, b * NB:(b + 1) * NB, 0:H], pv[:])
                    return
                if cfg['flip_v']:
                    # flipped: x-chunk stationary, Wv streamed -> natural [t, h]
                    ps = psA.tile([P, NB, H], f32, tag="a", name="ps_v")
                    for s in range(NB):
                        t = b * NB + s
                        tsl = slice(t * P, (t + 1) * P)
                        for c in range(CC):
                            nc.tensor.matmul(
                                ps[:, s, :], x_sb[:, c, tsl], wv_sb[:, c, :],
                                start=(c == 0), stop=(c == CC - 1),
                            )
                    nc.vector.tensor_copy(v_sb[:, b * NB:(b + 1) * NB, 0:H], ps[:])
                else:
                    # W-stationary: vT [64h, 512] then PE-transpose per 128-tile
                    bsl = slice(b * QB, (b + 1) * QB)
                    ps = psA.tile([P, QB], f32, tag="a", name="ps_vT")
                    for c in range(CC):
                        nc.tensor.matmul(
                            ps[0:H, :], wv_sb[:, c, :], x_sb[:, c, bsl],
                            start=(c == 0), stop=(c == CC - 1),
                        )
                    vT = otp.tile([H, QB], mmdt, name="vT")
                    nc.vector.tensor_copy(vT, ps[0:H, :])
                    pv = psA.tile([P, NB, H], mmdt, tag="a", name="ps_vn")
                    for s in range(NB):
                        nc.tensor.matmul(
                            pv[:, s, :],
                            vT[:, s * P:(s + 1) * P],
                            ident_mm[:H, :H],
                            is_transpose=True,
                        )
                    nc.vector.tensor_copy(v_sb[:, b * NB:(b + 1) * NB, 0:H], pv[:])

            def attn_scores(b, q_sb, k_sb, weave=None):
                # weave: (ets_prev, v_sb, out_sb) -> emit block b-1's att@v
                # chunks between score pairs so the PE fills its exp/psum
                # throttle waits with ready work
                nk = (b + 1) * 4
                if weave is not None:
                    ets_prev, wv, wout = weave
                    po_prev = psO.tile([H + 1, QB], f32, tag="o", name="ps_o")
                    nk_prev = 4 * b
                    ci = 0
                if cfg['chunked']:
                    # per-chunk 1-bank score tiles + per-chunk exp: deeper
                    # scores->exp pipeline (psS_bufs slots instead of 2 pair
                    # slots) at the cost of 2x ACT call overhead
                    ets = []
                    et2 = None
                    for kc in range(nk):
                        s = kc - 4 * b
                        lo = s * P if s > 0 else 0
                        ps1 = psS.tile([P, QB], f32, tag="s", name="ps_s")
                        half = (kc % 2) * H
                        nc.tensor.matmul(
                            ps1[:, lo:QB],
                            k_sb[half:half + H, (kc // 2) * P:(kc // 2 + 1) * P],
                            q_sb[half:half + H, lo:QB],
                            tile_position=(half, 0),
                        )
                        if kc % 2 == 0:
                            et2 = expp.tile([P, 2, QB], mmdt, tag="e", name="et")
                            ets.append(et2)
                        if not cfg['no_act']:
                            nc.scalar.activation(
                                et2[:, kc % 2, lo:QB], ps1[:, lo:QB], AF.Exp)
                            if s >= 0:
                                eng = nc.vector if cfg['mask_dve'] else nc.gpsimd
                                eng.tensor_mul(
                                    et2[:, kc % 2, s * P:(s + 1) * P],
                                    et2[:, kc % 2, s * P:(s + 1) * P],
                                    mask128[:, :],
                                )
                    return ets
                npair = (nk + 1) // 2
                ets = []
                for j in range(npair):
                    ps2 = psS.tile([P, 2, QB], f32, tag="s", name="ps_s")
                    et2 = expp.tile([P, 2, QB], mmdt, tag="e", name="et")
                    for jj in range(2):
                        kc = 2 * j + jj
                        if kc >= nk:
                            continue
                        s = kc - 4 * b  # diag sub-block index if >= 0
                        lo = s * P if s > 0 else 0
                        pm = mybir.MatmulPerfMode.DoublePixel if cfg['dpix'] else None
                        if cfg['alt_scores']:
                            half = (kc % 2) * H
                            nc.tensor.matmul(
                                ps2[:, jj, lo:QB],
                                k_sb[half:half + H, (kc // 2) * P:(kc // 2 + 1) * P],
                                q_sb[half:half + H, lo:QB],
                                tile_position=(half, 0),
                                perf_mode=pm,
                            )
                        else:
                            nc.tensor.matmul(
                                ps2[:, jj, lo:QB],
                                k_sb[:, kc * P:(kc + 1) * P],
                                q_sb[:, lo:QB],
                                perf_mode=pm,
                            )
                    if not cfg['no_act']:
                        lo_pair = 2 * P if 2 * j - 4 * b == 2 else 0
                        nc.scalar.activation(
                            et2[:, :, lo_pair:QB], ps2[:, :, lo_pair:QB], AF.Exp)
                    for jj in range(2):
                        kc = 2 * j + jj
                        if kc >= nk:
                            continue
                        s = kc - 4 * b
                        if s >= 0 and not cfg['no_act']:
                            eng = nc.vector if cfg['mask_dve'] else nc.gpsimd
                            eng.tensor_mul(
                                et2[:, jj, s * P:(s + 1) * P],
                                et2[:, jj, s * P:(s + 1) * P],
                                mask128[:, :],
                            )
                    ets.append(et2)
                    if weave is not None:
                        want = (nk_prev * (j + 1) + npair - 1) // npair
                        while ci < want:
                            attv_chunk(b - 1, ci, ets_prev, wv, po_prev)
                            ci += 1
                if weave is not None:
                    attn_tail(b - 1, po_prev, wout)
                return ets

            def attv_chunk(b, kc, ets, v_sb, po):
                nk = (b + 1) * 4
                s = kc - 4 * b
                lo = s * P if s > 0 else 0
                nc.tensor.matmul(
                    po[:, lo:QB],
                    v_sb[:, kc, :],
                    (et_const if cfg['no_act'] else ets[kc // 2])[:, kc % 2, lo:QB],
                    start=(kc == 0),
                    stop=(kc == nk - 1),
                )

            def attn_out(b, ets, v_sb, out_sb):
                nk = (b + 1) * 4
                # att @ [v | 1]: rows 0..63 = out^T, row 64 = softmax denom
                po = psO.tile([H + 1, QB], f32, tag="o", name="ps_o")
                for kc in range(nk):
                    attv_chunk(b, kc, ets, v_sb, po)
                attn_tail(b, po, out_sb)

            def attn_tail(b, po, out_sb):
                if cfg['no_tail']:
                    return
                odt = bf16 if cfg['bf16_out'] else f32
                oT = otp.tile([H + 1, QB], odt, name="oT")
                if cfg['act_ot']:
                    # Act shares the exp table set with 'copy' (no table
                    # reload): offload this psum evacuation from the busy DVE
                    nc.scalar.activation(oT, po, AF.Copy)
                else:
                    nc.vector.tensor_copy(oT, po)

                # all 4 transposes into one psum tile, then one batched
                # reciprocal: avoids a 4-deep PE<->DVE ping-pong on the pool
                pt4 = psO.tile([P, NB, H + 2], odt, tag="o", name="ps_t4")
                for s in range(NB):
                    nc.tensor.matmul(
                        pt4[:, s, 0:H + 1],
                        oT[:, s * P:(s + 1) * P],
                        (ident_mm if cfg['bf16_out'] else ident)[:H + 1, :H + 1],
                        is_transpose=True,
                    )
                rc4 = sclp.tile([P, NB], f32, name="rc4")
                nc.vector.reciprocal(rc4, pt4[:, :, H])
                for s in range(NB):
                    t = b * NB + s
                    nc.vector.tensor_scalar_mul(
                        out_sb[:, t, :], pt4[:, s, 0:H], rc4[:, s:s + 1])

                if not cfg['dma1']:
                    nc.sync.dma_start(
                        out_d[:, :].rearrange("(g p) h -> p g h", p=P)[:, b * 4:(b + 1) * 4, :],
                        out_sb[:, b * 4:(b + 1) * 4, :],
                    )
                elif b == NB - 1:
                    nc.sync.dma_start(
                        out_d[:, :].rearrange("(g p) h -> p g h", p=P),
                        out_sb[:, :, :],
                    )

            def proj(b, k_sb, v_sb):
                if cfg['mproj']:
                    return project_mqkv(b, k_sb, v_sb)
                if cfg['vweave']:
                    return project_qk(b, k_sb, v_sb=v_sb)
                q_sb = project_qk(b, k_sb)
                project_v(b, v_sb)
                return q_sb

            def body(prev=None):
                _, k_sb, v_sb, out_sb = make_bufs()
                if cfg['vlate']:
                    # v(b) only feeds attn_out(b) an iteration later: emit it
                    # after scores(b) so exp work is released ~1us earlier and
                    # the LDW-heavy v matmuls fill PE while ACT drains exps.
                    # With rotate, attn_out(3) of the previous rep is deferred
                    # past qk(0)+scores(0) so PE isn't stuck behind block 3's
                    # ~8us exp chain at the rep boundary.
                    q_sb = project_qk(0, k_sb)
                    if cfg['rotate']:
                        ets = attn_scores(0, q_sb, k_sb)
                        if prev is not None:
                            attn_out(NB - 1, *prev)
                        project_v(0, v_sb)
                    else:
                        if prev is not None:
                            attn_out(NB - 1, *prev)
                        ets = attn_scores(0, q_sb, k_sb)
                        project_v(0, v_sb)
                    for b in range(1, NB):
                        q_sb = project_qk(b, k_sb)
                        attn_out(b - 1, ets, v_sb, out_sb)
                        ets = attn_scores(b, q_sb, k_sb)
                        project_v(b, v_sb)
                    if cfg['rotate']:
                        return (ets, v_sb, out_sb)
                    attn_out(NB - 1, ets, v_sb, out_sb)
                    return None
                if prev is not None:
                    attn_out(NB - 1, *prev)
                q_sb = proj(0, k_sb, v_sb)
                ets = attn_scores(0, q_sb, k_sb)
                for b in range(1, NB):
                    q_sb = proj(b, k_sb, v_sb)
                    if cfg['weave']:
                        ets = attn_scores(b, q_sb, k_sb,
                                          weave=(ets, v_sb, out_sb))
                    elif cfg['early_scores']:
                        ets_new = attn_scores(b, q_sb, k_sb)
                        attn_out(b - 1, ets, v_sb, out_sb)
                        ets = ets_new
                    else:
                        attn_out(b - 1, ets, v_sb, out_sb)
                        ets = attn_scores(b, q_sb, k_sb)
                if cfg['rotate']:
                    return (ets, v_sb, out_sb)
                attn_out(NB - 1, ets, v_sb, out_sb)
                return None

            if cfg['no_tail']:
                dummy_out = outp.tile([P, KT, H], bf16 if cfg['bf16_dma'] else f32, name="dummy_out")
                nc.gpsimd.memset(dummy_out[:], 0.0)
                nc.sync.dma_start(
                    out_d[:, :].rearrange("(g p) h -> p g h", p=P), dummy_out[:])

            if outer:
                with tc.For_i(0, outer):
                    prev = None
                    for _rep in range(reps):
                        prev = body(prev)
                    if prev is not None:
                        attn_out(NB - 1, *prev)
            else:
                prev = None
                for _rep in range(reps):
                    prev = body(prev)
                if prev is not None:
                    attn_out(NB - 1, *prev)

    nc.compile()
    return nc


def _get_nc():
    nc = _CACHE.get("nc")
    if nc is None:
        nc = _build()
        _CACHE["nc"] = nc
    return nc


def _make_in_maps(inputs, cfg=None):
    cfg = dict(CFG, **(cfg or {}))
    if cfg['bf16']:
        from ml_dtypes import bfloat16
        mdt = bfloat16
    else:
        mdt = np.float32
    x = np.asarray(inputs["x"], dtype=np.float32)
    Wq = np.asarray(inputs["Wq"], dtype=np.float32)
    Wk = np.asarray(inputs["Wk"], dtype=np.float32)
    Wv = np.asarray(inputs["Wv"], dtype=np.float32)
    scale = np.float32(1.0 / np.sqrt(np.float32(Wq.shape[1])))
    wparts = [Wq * scale, Wk] + ([Wv] if cfg['mproj'] else [])
    wqk = np.ascontiguousarray(np.concatenate(wparts, axis=1)).astype(mdt)
    wv_c = np.ascontiguousarray(Wv).astype(mdt)
    in_maps = []
    for b in range(N_CORES):
        in_maps.append({
            "xT": np.ascontiguousarray(x[b].T).astype(mdt),
            "wqk": wqk,
            "wv": wv_c,
        })
    return in_maps


def _run(inputs, **kwargs):
    from concourse.bass_utils import run_bass_kernel_spmd

    nc = _get_nc()
    res = run_bass_kernel_spmd(nc, _make_in_maps(inputs), core_ids=list(range(N_CORES)), **kwargs)
    out = np.stack([res.results[i]["out"] for i in range(N_CORES)], axis=0)
    return out.astype(np.float32, copy=False), res


def kernel(**inputs):
    out, _ = _run(inputs)
    return out


def kernel_profiled(**inputs):
    """Returns (out, BassKernelResults); exec_time_ns only if tracing works."""
    out, res = _run(inputs)
    return out, res



# revision 24
# speedup vs baseline: 1.0060x; 1.0060x over previous
"""Trainium2 Bass kernel: single-head causal attention.

Problem: x[B=8,T=2048,C=1024] @ Wq/Wk/Wv[C,H=64] -> causal softmax attention
-> out[B,T,H].  Sharding: pure data-parallel over B, one batch element per
NeuronCore (8 cores, no collectives).

Layout strategy (per core):
  - host feeds x[b].T  (so the C contraction dim lands on SBUF partitions)
  - q,k projections: W-chunk stationary ([Wq*scale | Wk] packed to 128 cols)
    -> psum [128(q|k), 512]; one DVE copy puts q^T on partitions 0-63 of
    q_sb and one puts k^T on partitions 0-63 of k_sb
  - v projection: x-chunk stationary (flip_v) -> psum [128t, 64] lands v in
    natural [T,64] layout directly, no transposes (this beat W-stationary +
    PE transposes by ~4-6us measured); v_sb carries a ones column so att@v
    and the softmax row-sums come out of one accumulated matmul
  - scores are computed in sT layout [T_k, T_q]; diagonal 128-blocks are
    shortened to their live q-range; softmax uses exp without max-subtraction
    (|s| <~ 6 so fp32 exp is safe); causal mask is a single [128,128]
    staircase multiply on the boundary sub-block only (gpsimd)
  - score matmuls have 64-deep contraction, so k-chunks alternate PE row
    halves (tile_position) and pairs overlap in the array (~3us measured)
  - outT_aug [65, T_q] is evacuated on DVE (ACT copy measured slower once
    exp saturates ACT) and PE-transposed to [T_q, 65] per 128-row subtile;
    rows are scaled by 1/sum and DMA'd out per q-block (dma1=0)
  - all matmuls in bf16 (rel err 5.4e-3 vs 2e-2 budget)
  - q/k/v/out SBUF buffers are double-buffered; the last q-block's att@v
    runs at the start of the next rep (prev mechanism) to overlap its tail

Measured (differential For_i timing, 8 cores): 41.4us (session start,
W-stationary v) -> ~30.4us (flip_v + act_ot:0 + dma1:0).  Rejected by
measurement: mproj (+15us), vweave (+2us), weave/early_scores (+1-6us),
vpack (verifier reject), mask_dve, rotate, dve_dup:0 (+5us), expp 20,
chunked per-chunk score psum (+5us), rev diag-first pair order (+1us),
head2 2-pair ACT head start (+4us), vlate (+3us).  Every manual
reordering loses: the Tile scheduler's cost-model static schedule is
already gapless on PE (verified via CoreSim no_exec get_inst_timings);
residual HW-over-model time is LDWEIGHTS + sem-timing slack that
ordering cannot fix.  Cost-model floor ~25.5us PE-busy; ACT exp ~19us
busy (not critical: no_act only saves 2us).
"""

import numpy as np

P = 128
B = 8
T = 2048
C = 1024
H = 64
QB = 512          # q-block width (score tile free dim)
NB = T // QB      # 4 q-blocks
CC = C // P       # 8 contraction chunks
KT = T // P       # 16 key tiles / T subtiles
N_CORES = 8

_CACHE = {}
CFG = {'flip_v': True, 'alt_scores': True, 'no_act': False, 'no_proj': False,
       'no_tail': False, 'bf16': True, 'dpix': False, 'bf16_out': True,
       'rotate': False, 'dma1': False, 'kbatch': True, 'mproj': False, 'bf16_dma': True, 'early_scores': False, 'weave': False, 'act_ot': False, 'dve_dup': True,
       'vpack': False, 'mask_dve': False, 'expp_bufs': 16, 'psS_bufs': 2,
       'psA_bufs': 2, 'psO_bufs': 2, 'vweave': False, 'vlate': False,
       'chunked': False, 'rev': False, 'head2': False}


def _build(reps=1, outer=0, cfg=None):
    import concourse.bacc as bacc
    import concourse.mybir as mybir
    import concourse.tile as tile
    from concourse.masks import make_identity

    cfg = dict(CFG, **(cfg or {}))
    dt = mybir.dt
    f32 = dt.float32
    f32r = dt.float32r
    bf16 = dt.bfloat16
    AF = mybir.ActivationFunctionType
    ALU = mybir.AluOpType

    mmdt = bf16 if CFG['bf16'] else f32r
    nc = bacc.Bacc(None, target_bir_lowering=False)
    xT_d = nc.dram_tensor("xT", [C, T], mmdt, kind="ExternalInput")
    nw = 3 * H if cfg['mproj'] else 2 * H
    wqk_d = nc.dram_tensor("wqk", [C, nw], mmdt, kind="ExternalInput")
    wv_d = nc.dram_tensor("wv", [C, H], mmdt, kind="ExternalInput")
    out_d = nc.dram_tensor("out", [T, H], bf16 if cfg['bf16_dma'] else f32,
                           kind="ExternalOutput")

    with tile.TileContext(nc) as tc:
        with (
            tc.tile_pool(name="consts", bufs=1) as consts,
            tc.tile_pool(name="xpool", bufs=1) as xpool,
            tc.tile_pool(name="qp", bufs=2) as qp,
            tc.tile_pool(name="kp", bufs=2) as kp,
            tc.tile_pool(name="vp", bufs=2) as vp,
            tc.tile_pool(name="expp", bufs=cfg['expp_bufs']) as expp,
            tc.tile_pool(name="otp", bufs=3) as otp,
            tc.tile_pool(name="sclp", bufs=4) as sclp,
            tc.tile_pool(name="outp", bufs=2) as outp,
            tc.tile_pool(name="psA", bufs=cfg['psA_bufs'], space="PSUM") as psA,
            tc.tile_pool(name="psS", bufs=cfg['psS_bufs'], space="PSUM") as psS,
            tc.tile_pool(name="psO", bufs=cfg['psO_bufs'], space="PSUM") as psO,
        ):
            ident = consts.tile([P, P], f32)
            make_identity(nc, ident)
            ident_mm = consts.tile([P, P], mmdt)
            nc.vector.tensor_copy(ident_mm, ident)
            # mask128[p, f] = 1.0 if f >= p else 0.0 (staircase for the
            # boundary 128x128 sub-block of each diagonal score tile)
            mask128 = consts.tile([P, P], mmdt)
            nc.gpsimd.memset(mask128, 1.0)
            nc.gpsimd.affine_select(
                out=mask128,
                in_=mask128,
                compare_op=ALU.is_ge,
                fill=0.0,
                base=0,
                pattern=[[1, P]],
                channel_multiplier=-1,
            )

            wqk_sb = consts.tile([P, CC, nw], mmdt)
            nc.sync.dma_start(wqk_sb[:], wqk_d[:, :].rearrange("(c p) h -> p c h", p=P))
            wv_sb = consts.tile([P, CC, H], mmdt)
            nc.sync.dma_start(wv_sb[:], wv_d[:, :].rearrange("(c p) h -> p c h", p=P))

            x_sb = xpool.tile([P, CC, T], mmdt)
            for bb in range(NB // 2):
                for c in range(CC):
                    nc.sync.dma_start(
                        x_sb[:, c, bb * 2 * QB:(bb + 1) * 2 * QB],
                        xT_d[c * P:(c + 1) * P, bb * 2 * QB:(bb + 1) * 2 * QB],
                    )

            ones_col = consts.tile([P, KT, 1], f32)
            nc.gpsimd.memset(ones_col[:], 1.0)

            if cfg['no_act']:
                et_const = consts.tile([P, 2, QB], mmdt)
                scr = consts.tile([P, 2, QB], f32)
                nc.gpsimd.memset(scr[:], 0.001)
                nc.vector.tensor_copy(et_const[:], scr[:])

            if cfg['no_proj']:
                scr2 = consts.tile([P, T // 2], f32)
                nc.gpsimd.memset(scr2[:], 0.01)
                qC = consts.tile([P, QB], mmdt)
                kC = consts.tile([P, T // 2], mmdt)
                vC = consts.tile([P, KT, H + 1], mmdt)
                nc.vector.tensor_copy(qC[:], scr2[:, 0:QB])
                nc.vector.tensor_copy(kC[:], scr2[:])
                for _t in range(KT):
                    nc.vector.tensor_copy(vC[:, _t, :], scr2[:, 0:H + 1])

            def make_bufs():
                if cfg['no_proj']:
                    out_sb = outp.tile([P, KT, H], bf16 if cfg['bf16_dma'] else f32, name="out_sb")
                    return None, kC, vC, out_sb
                # per-iteration rotating state: k history, v history
                if cfg['alt_scores']:
                    k_sb = kp.tile([P, T // 2], mmdt, name="k_sb")
                else:
                    k_sb = kp.tile([H, T], mmdt, name="k_sb")
                v_sb = vp.tile([P, KT, H + 1], mmdt, name="v_sb")
                nc.vector.tensor_copy(v_sb[:, :, H:H + 1], ones_col[:])
                out_sb = outp.tile([P, KT, H], bf16 if cfg['bf16_dma'] else f32, name="out_sb")
                return None, k_sb, v_sb, out_sb

            def project_mqkv(b, k_sb, v_sb):
                # one pass over x: q,k,v in natural [t, 3H] layout, then PE
                # transposes for q,k; v slices feed v_sb directly
                q_sb = qp.tile([P, QB], mmdt, name="q_sb")
                for r in range(2):
                    ps = psA.tile([P, 2, 3 * H], f32, tag="a", name="ps_m")
                    for tt in range(2):
                        t = b * NB + 2 * r + tt
                        tsl = slice(t * P, (t + 1) * P)
                        for c in range(CC):
                            nc.tensor.matmul(
                                ps[:, tt, :], x_sb[:, c, tsl], wqk_sb[:, c, :],
                                start=(c == 0), stop=(c == CC - 1),
                            )
                    nat = otp.tile([P, 2, 3 * H], mmdt, name="nat")
                    nc.vector.tensor_copy(nat, ps)
                    # v: natural already; strided copy into v_sb
                    nc.vector.tensor_copy(
                        v_sb[:, b * NB + 2 * r:b * NB + 2 * r + 2, 0:H],
                        nat[:, :, 2 * H:3 * H],
                    )
                    for tt in range(2):
                        t2 = 2 * r + tt
                        kc = 4 * b + t2
                        pq = psA.tile([H, 2, P], mmdt, tag="a", name="ps_tq")
                        nc.tensor.matmul(
                            pq[:, 0, :], nat[:, tt, 0:H],
                            ident_mm[:P, :P], is_transpose=True,
                        )
                        nc.tensor.matmul(
                            pq[:, 1, :], nat[:, tt, H:2 * H],
                            ident_mm[:P, :P], is_transpose=True,
                        )
                        nc.vector.tensor_copy(
                            q_sb[0:H, t2 * P:(t2 + 1) * P], pq[:, 0, :])
                        half = (kc % 2) * H
                        nc.vector.tensor_copy(
                            k_sb[half:half + H, (kc // 2) * P:(kc // 2 + 1) * P],
                            pq[:, 1, :],
                        )
                nc.gpsimd.tensor_copy(q_sb[H:P, :], q_sb[0:H, :])
                return q_sb

            def project_qk(b, k_sb, v_sb=None):
                if cfg['no_proj']:
                    return qC
                if cfg['alt_scores']:
                    q_sb = qp.tile([P, QB], mmdt, name="q_sb")
                else:
                    q_sb = qp.tile([H, QB], mmdt, name="q_sb")
                # [Wq*s|Wk] concatenated on host -> one M=128 matmul gives
                # qT on psum parts 0-63 and kT on parts 64-127
                bsl = slice(b * QB, (b + 1) * QB)
                ps = psA.tile([P, QB], f32, tag="a", name="ps_qk")
                psv = None
                if v_sb is not None:
                    # vweave: interleave the 32 x-stationary v matmuls (64-col
                    # streams) between the 8 long qk streams so each v
                    # LDWEIGHTS overlaps a 512-col qk matmul
                    psv = psA.tile([P, NB, H], f32, tag="a", name="ps_v")
                for c in range(CC):
                    nc.tensor.matmul(
                        ps, wqk_sb[:, c, :], x_sb[:, c, bsl],
                        start=(c == 0), stop=(c == CC - 1),
                    )
                    if psv is not None:
                        for s in range(NB):
                            t = b * NB + s
                            tsl = slice(t * P, (t + 1) * P)
                            nc.tensor.matmul(
                                psv[:, s, :], x_sb[:, c, tsl], wv_sb[:, c, :],
                                start=(c == 0), stop=(c == CC - 1),
                            )
                if psv is not None:
                    nc.vector.tensor_copy(
                        v_sb[:, b * NB:(b + 1) * NB, 0:H], psv[:])
                if cfg['alt_scores']:
                    # q duplicated on both partition halves; k chunks go to
                    # alternating halves so score matmuls ping-pong PE rows
                    nc.vector.tensor_copy(q_sb[0:H, :], ps[0:H, :])
                    if cfg['dve_dup']:
                        # all-bf16 SBUF copy hits DVE 4x mode and lands on an
                        # empty queue; on gpsimd it sat behind the mask queue
                        nc.vector.tensor_copy(q_sb[H:P, :], q_sb[0:H, :])
                    else:
                        nc.gpsimd.tensor_copy(q_sb[H:P, :], q_sb[0:H, :])
                    if cfg['kbatch']:
                        s4 = ps[H:P, :].rearrange("p (a k) -> p a k", k=P)
                        d4 = k_sb[:, 2 * b * P:(2 * b + 2) * P].rearrange(
                            "p (a k) -> p a k", k=P)
                        nc.vector.tensor_copy(d4[0:H, :, :], s4[:, 0::2, :])
                        nc.vector.tensor_copy(d4[H:P, :, :], s4[:, 1::2, :])
                    else:
                        for s in range(4):
                            kc = 4 * b + s
                            half = (kc % 2) * H
                            nc.vector.tensor_copy(
                                k_sb[half:half + H, (kc // 2) * P:(kc // 2 + 1) * P],
                                ps[H:P, s * P:(s + 1) * P],
                            )
                else:
                    nc.vector.tensor_copy(q_sb[:, :], ps[0:H, :])
                    nc.vector.tensor_copy(k_sb[:, bsl], ps[H:P, :])
                return q_sb

            def project_v(b, v_sb):
                if cfg['no_proj']:
                    return
                if cfg['vpack']:
                    # col-packed W-stationary: two c-chunks' Wv in disjoint
                    # col groups (partitions 0-63 / 64-127) stream their x
                    # chunks concurrently; halves summed on DVE afterwards
                    bsl = slice(b * QB, (b + 1) * QB)
                    ps = psA.tile([P, QB], f32, tag="a", name="ps_vp")
                    for cp in range(CC // 2):
                        nc.tensor.matmul(
                            ps[0:H, :], wv_sb[:, 2 * cp, :], x_sb[:, 2 * cp, bsl],
                            start=(cp == 0), stop=(cp == CC // 2 - 1),
                            tile_position=(0, 0),
                        )
                        nc.tensor.matmul(
                            ps[H:2 * H, :], wv_sb[:, 2 * cp + 1, :],
                            x_sb[:, 2 * cp + 1, bsl],
                            start=(cp == 0), stop=(cp == CC // 2 - 1),
                            tile_position=(0, 64),
                        )
                    vT = otp.tile([H, QB], mmdt, name="vT")
                    nc.vector.tensor_add(vT, ps[0:H, :], ps[H:2 * H, :])
                    pv = psA.tile([P, NB, H], mmdt, tag="a", name="ps_vn")
                    for s in range(NB):
                        nc.tensor.matmul(
                            pv[:, s, :],
                            vT[:, s * P:(s + 1) * P],
                            ident_mm[:H, :H],
                            is_transpose=True,
                        )
                    nc.vector.tensor_copy(v_sb[:, b * NB:(b + 1) * NB, 0:H], pv[:])
                    return
                if cfg['flip_v']:
                    # flipped: x-chunk stationary, Wv streamed -> natural [t, h]
                    ps = psA.tile([P, NB, H], f32, tag="a", name="ps_v")
                    for s in range(NB):
                        t = b * NB + s
                        tsl = slice(t * P, (t + 1) * P)
                        for c in range(CC):
                            nc.tensor.matmul(
                                ps[:, s, :], x_sb[:, c, tsl], wv_sb[:, c, :],
                                start=(c == 0), stop=(c == CC - 1),
                            )
                    nc.vector.tensor_copy(v_sb[:, b * NB:(b + 1) * NB, 0:H], ps[:])
                else:
                    # W-stationary: vT [64h, 512] then PE-transpose per 128-tile
                    bsl = slice(b * QB, (b + 1) * QB)
                    ps = psA.tile([P, QB], f32, tag="a", name="ps_vT")
                    for c in range(CC):
                        nc.tensor.matmul(
                            ps[0:H, :], wv_sb[:, c, :], x_sb[:, c, bsl],
                            start=(c == 0), stop=(c == CC - 1),
                        )
                    vT = otp.tile([H, QB], mmdt, name="vT")
                    nc.vector.tensor_copy(vT, ps[0:H, :])
                    pv = psA.tile([P, NB, H], mmdt, tag="a", name="ps_vn")
                    for s in range(NB):
                        nc.tensor.matmul(
                            pv[:, s, :],
                            vT[:, s * P:(s + 1) * P],
                            ident_mm[:H, :H],
                            is_transpose=True,
                        )
                    nc.vector.tensor_copy(v_sb[:, b * NB:(b + 1) * NB, 0:H], pv[:])

            def attn_scores(b, q_sb, k_sb, weave=None, pairs=None, ets=None):
                # weave: (ets_prev, v_sb, out_sb) -> emit block b-1's att@v
                # chunks between score pairs so the PE fills its exp/psum
                # throttle waits with ready work
                # pairs/ets: emit only the given pair indices into ets (split
                # emission around attn_out); cfg rev reverses pair order
                nk = (b + 1) * 4
                if weave is not None:
                    ets_prev, wv, wout = weave
                    po_prev = psO.tile([H + 1, QB], f32, tag="o", name="ps_o")
                    nk_prev = 4 * b
                    ci = 0
                if cfg['chunked']:
                    # per-chunk 1-bank score tiles + per-chunk exp: deeper
                    # scores->exp pipeline (psS_bufs slots instead of 2 pair
                    # slots) at the cost of 2x ACT call overhead
                    ets = []
                    et2 = None
                    for kc in range(nk):
                        s = kc - 4 * b
                        lo = s * P if s > 0 else 0
                        ps1 = psS.tile([P, QB], f32, tag="s", name="ps_s")
                        half = (kc % 2) * H
                        nc.tensor.matmul(
                            ps1[:, lo:QB],
                            k_sb[half:half + H, (kc // 2) * P:(kc // 2 + 1) * P],
                            q_sb[half:half + H, lo:QB],
                            tile_position=(half, 0),
                        )
                        if kc % 2 == 0:
                            et2 = expp.tile([P, 2, QB], mmdt, tag="e", name="et")
                            ets.append(et2)
                        if not cfg['no_act']:
                            nc.scalar.activation(
                                et2[:, kc % 2, lo:QB], ps1[:, lo:QB], AF.Exp)
                            if s >= 0:
                                eng = nc.vector if cfg['mask_dve'] else nc.gpsimd
                                eng.tensor_mul(
                                    et2[:, kc % 2, s * P:(s + 1) * P],
                                    et2[:, kc % 2, s * P:(s + 1) * P],
                                    mask128[:, :],
                                )
                    return ets
                npair = (nk + 1) // 2
                if pairs is None:
                    pairs = list(range(npair))
                    if cfg['rev']:
                        # diag pairs first: their masks clear early and the
                        # block's last attv chunk has no gpsimd mask hop
                        pairs = pairs[::-1]
                if ets is None:
                    ets = {}
                for j in pairs:
                    ps2 = psS.tile([P, 2, QB], f32, tag="s", name="ps_s")
                    et2 = expp.tile([P, 2, QB], mmdt, tag="e", name="et")
                    for jj in range(2):
                        kc = 2 * j + jj
                        if kc >= nk:
                            continue
                        s = kc - 4 * b  # diag sub-block index if >= 0
                        lo = s * P if s > 0 else 0
                        pm = mybir.MatmulPerfMode.DoublePixel if cfg['dpix'] else None
                        if cfg['alt_scores']:
                            half = (kc % 2) * H
                            nc.tensor.matmul(
                                ps2[:, jj, lo:QB],
                                k_sb[half:half + H, (kc // 2) * P:(kc // 2 + 1) * P],
                                q_sb[half:half + H, lo:QB],
                                tile_position=(half, 0),
                                perf_mode=pm,
                            )
                        else:
                            nc.tensor.matmul(
                                ps2[:, jj, lo:QB],
                                k_sb[:, kc * P:(kc + 1) * P],
                                q_sb[:, lo:QB],
                                perf_mode=pm,
                            )
                    if not cfg['no_act']:
                        lo_pair = 2 * P if 2 * j - 4 * b == 2 else 0
                        nc.scalar.activation(
                            et2[:, :, lo_pair:QB], ps2[:, :, lo_pair:QB], AF.Exp)
                    for jj in range(2):
                        kc = 2 * j + jj
                        if kc >= nk:
                            continue
                        s = kc - 4 * b
                        if s >= 0 and not cfg['no_act']:
                            eng = nc.vector if cfg['mask_dve'] else nc.gpsimd
                            eng.tensor_mul(
                                et2[:, jj, s * P:(s + 1) * P],
                                et2[:, jj, s * P:(s + 1) * P],
                                mask128[:, :],
                            )
                    ets[j] = et2
                    if weave is not None:
                        want = (nk_prev * (len(ets)) + npair - 1) // npair
                        while ci < want:
                            attv_chunk(b - 1, ci, ets_prev, wv, po_prev,
                                       start=(ci == 0), stop=(ci == nk_prev - 1))
                            ci += 1
                if weave is not None:
                    attn_tail(b - 1, po_prev, wout)
                return ets

            def attv_chunk(b, kc, ets, v_sb, po, start, stop):
                nk = (b + 1) * 4
                s = kc - 4 * b
                lo = s * P if s > 0 else 0
                nc.tensor.matmul(
                    po[:, lo:QB],
                    v_sb[:, kc, :],
                    (et_const if cfg['no_act'] else ets[kc // 2])[:, kc % 2, lo:QB],
                    start=start,
                    stop=stop,
                )

            def attn_out(b, ets, v_sb, out_sb):
                nk = (b + 1) * 4
                npair = (nk + 1) // 2
                # att @ [v | 1]: rows 0..63 = out^T, row 64 = softmax denom
                # chunk order matches the score-pair issue order so attv can
                # start on the earliest-exp'd pair
                po = psO.tile([H + 1, QB], f32, tag="o", name="ps_o")
                pairs = list(range(npair))
                if cfg['rev']:
                    pairs = pairs[::-1]
                kcs = [2 * j + jj for j in pairs for jj in range(2) if 2 * j + jj < nk]
                for i, kc in enumerate(kcs):
                    attv_chunk(b, kc, ets, v_sb, po,
                               start=(i == 0), stop=(i == len(kcs) - 1))
                attn_tail(b, po, out_sb)

            def attn_tail(b, po, out_sb):
                if cfg['no_tail']:
                    return
                odt = bf16 if cfg['bf16_out'] else f32
                oT = otp.tile([H + 1, QB], odt, name="oT")
                if cfg['act_ot']:
                    # Act shares the exp table set with 'copy' (no table
                    # reload): offload this psum evacuation from the busy DVE
                    nc.scalar.activation(oT, po, AF.Copy)
                else:
                    nc.vector.tensor_copy(oT, po)

                # all 4 transposes into one psum tile, then one batched
                # reciprocal: avoids a 4-deep PE<->DVE ping-pong on the pool
                pt4 = psO.tile([P, NB, H + 2], odt, tag="o", name="ps_t4")
                for s in range(NB):
                    nc.tensor.matmul(
                        pt4[:, s, 0:H + 1],
                        oT[:, s * P:(s + 1) * P],
                        (ident_mm if cfg['bf16_out'] else ident)[:H + 1, :H + 1],
                        is_transpose=True,
                    )
                rc4 = sclp.tile([P, NB], f32, name="rc4")
                nc.vector.reciprocal(rc4, pt4[:, :, H])
                for s in range(NB):
                    t = b * NB + s
                    nc.vector.tensor_scalar_mul(
                        out_sb[:, t, :], pt4[:, s, 0:H], rc4[:, s:s + 1])

                if not cfg['dma1']:
                    nc.sync.dma_start(
                        out_d[:, :].rearrange("(g p) h -> p g h", p=P)[:, b * 4:(b + 1) * 4, :],
                        out_sb[:, b * 4:(b + 1) * 4, :],
                    )
                elif b == NB - 1:
                    nc.sync.dma_start(
                        out_d[:, :].rearrange("(g p) h -> p g h", p=P),
                        out_sb[:, :, :],
                    )

            def proj(b, k_sb, v_sb):
                if cfg['mproj']:
                    return project_mqkv(b, k_sb, v_sb)
                if cfg['vweave']:
                    return project_qk(b, k_sb, v_sb=v_sb)
                q_sb = project_qk(b, k_sb)
                project_v(b, v_sb)
                return q_sb

            def body(prev=None):
                _, k_sb, v_sb, out_sb = make_bufs()
                if cfg['vlate']:
                    # v(b) only feeds attn_out(b) an iteration later: emit it
                    # after scores(b) so exp work is released ~1us earlier and
                    # the LDW-heavy v matmuls fill PE while ACT drains exps.
                    # With rotate, attn_out(3) of the previous rep is deferred
                    # past qk(0)+scores(0) so PE isn't stuck behind block 3's
                    # ~8us exp chain at the rep boundary.
                    q_sb = project_qk(0, k_sb)
                    if cfg['rotate']:
                        ets = attn_scores(0, q_sb, k_sb)
                        if prev is not None:
                            attn_out(NB - 1, *prev)
                        project_v(0, v_sb)
                    else:
                        if prev is not None:
                            attn_out(NB - 1, *prev)
                        ets = attn_scores(0, q_sb, k_sb)
                        project_v(0, v_sb)
                    for b in range(1, NB):
                        q_sb = project_qk(b, k_sb)
                        attn_out(b - 1, ets, v_sb, out_sb)
                        ets = attn_scores(b, q_sb, k_sb)
                        project_v(b, v_sb)
                    if cfg['rotate']:
                        return (ets, v_sb, out_sb)
                    attn_out(NB - 1, ets, v_sb, out_sb)
                    return None
                if prev is not None:
                    attn_out(NB - 1, *prev)
                q_sb = proj(0, k_sb, v_sb)
                ets = attn_scores(0, q_sb, k_sb)
                for b in range(1, NB):
                    q_sb = proj(b, k_sb, v_sb)
                    if cfg['weave']:
                        ets = attn_scores(b, q_sb, k_sb,
                                          weave=(ets, v_sb, out_sb))
                    elif cfg['early_scores']:
                        ets_new = attn_scores(b, q_sb, k_sb)
                        attn_out(b - 1, ets, v_sb, out_sb)
                        ets = ets_new
                    elif cfg['head2']:
                        # give ACT a 2-pair head start on block b's exps
                        # before PE commits to block b-1's attv chunks
                        order = list(range(2 * (b + 1)))
                        if cfg['rev']:
                            order = order[::-1]
                        ets_new = attn_scores(b, q_sb, k_sb, pairs=order[:2])
                        attn_out(b - 1, ets, v_sb, out_sb)
                        attn_scores(b, q_sb, k_sb, pairs=order[2:], ets=ets_new)
                        ets = ets_new
                    else:
                        attn_out(b - 1, ets, v_sb, out_sb)
                        ets = attn_scores(b, q_sb, k_sb)
                if cfg['rotate']:
                    return (ets, v_sb, out_sb)
                attn_out(NB - 1, ets, v_sb, out_sb)
                return None

            if cfg['no_tail']:
                dummy_out = outp.tile([P, KT, H], bf16 if cfg['bf16_dma'] else f32, name="dummy_out")
                nc.gpsimd.memset(dummy_out[:], 0.0)
                nc.sync.dma_start(
                    out_d[:, :].rearrange("(g p) h -> p g h", p=P), dummy_out[:])

            if outer:
                with tc.For_i(0, outer):
                    prev = None
                    for _rep in range(reps):
                        prev = body(prev)
                    if prev is not None:
                        attn_out(NB - 1, *prev)
            else:
                prev = None
                for _rep in range(reps):
                    prev = body(prev)
                if prev is not None:
                    attn_out(NB - 1, *prev)

    nc.compile()
    return nc


def _get_nc():
    nc = _CACHE.get("nc")
    if nc is None:
        nc = _build()
        _CACHE["nc"] = nc
    return nc


def _make_in_maps(inputs, cfg=None):
    cfg = dict(CFG, **(cfg or {}))
    if cfg['bf16']:
        from ml_dtypes import bfloat16
        mdt = bfloat16
    else:
        mdt = np.float32
    x = np.asarray(inputs["x"], dtype=np.float32)
    Wq = np.asarray(inputs["Wq"], dtype=np.float32)
    Wk = np.asarray(inputs["Wk"], dtype=np.float32)
    Wv = np.asarray(inputs["Wv"], dtype=np.float32)
    scale = np.float32(1.0 / np.sqrt(np.float32(Wq.shape[1])))
    wparts = [Wq * scale, Wk] + ([Wv] if cfg['mproj'] else [])
    wqk = np.ascontiguousarray(np.concatenate(wparts, axis=1)).astype(mdt)
    wv_c = np.ascontiguousarray(Wv).astype(mdt)
    in_maps = []
    for b in range(N_CORES):
        in_maps.append({
            "xT": np.ascontiguousarray(x[b].T).astype(mdt),
            "wqk": wqk,
            "wv": wv_c,
        })
    return in_maps


def _run(inputs, **kwargs):
    from concourse.bass_utils import run_bass_kernel_spmd

    nc = _get_nc()
    res = run_bass_kernel_spmd(nc, _make_in_maps(inputs), core_ids=list(range(N_CORES)), **kwargs)
    out = np.stack([res.results[i]["out"] for i in range(N_CORES)], axis=0)
    return out.astype(np.float32, copy=False), res


def kernel(**inputs):
    out, _ = _run(inputs)
    return out


def kernel_profiled(**inputs):
    """Returns (out, BassKernelResults); exec_time_ns only if tracing works."""
    out, res = _run(inputs)
    return out, res

